# revision 42
# baseline (speedup 1.0000x reference)
"""Mexican-hat wavelet KAN layer + BatchNorm (training stats) on 8 TRN2 cores.

Reference computation (B=I=O=512):
    t   = (x[b,i] - bias[i,o]) / scale[i,o]
    wav = NORM * (t^2 - 1) * exp(-t^2/2)
    y   = einsum('bio,io->bo', wav, weight)
    out = batchnorm_train(y, gamma, beta)          # biased stats over batch

Fast path (scale/bias constant along O, which holds for the canonical
inputs): the affine (x-b)/s is folded into x on the host, so the device
computes u = x'^2, e = exp(-u/2), wav = (u-1)*e, y = wav^T @ w' with
MEXHAT_NORM folded into w'.  Sharding is data-parallel over the batch:
each core computes a 64-row batch slice of y for ALL 512 outputs (x slice
64KB + replicated weights 512KB, both fp16).  The BatchNorm epilogue (a
per-output affine from global batch stats) runs on the host over the
gathered y.  The fp16 datapath lands ~1e-3 max rel err, well inside the
2e-2 gate.

A numpy fallback evaluates the full per-(i,o) wavelet on the host when
the structure check fails (the canonical inputs never do).

The kernel is written in raw Bass (explicit semaphores, standalone wait_ge
instructions) because this walrus codegen caps every instruction at ONE
sync-wait: Tile's auto-semaphores attach multiple waits to one instruction
and fail to compile.
"""

import math
import os

import numpy as np

import concourse.bass as bass
from concourse import mybir
from concourse.bass_utils import run_bass_kernel_spmd

B, I, O = 512, 512, 512
N_CORES = 8
BS = B // N_CORES          # 64 batch rows per core (fast path)
KP = 128                   # partition chunk of the contraction dim
NK = I // KP               # 4 chunks
MEXHAT_NORM = 2.0 / (math.sqrt(3.0) * math.pi**0.25)
BN_EPS = 1e-5
FP32 = mybir.dt.float32
FP16 = mybir.dt.float16
F = mybir.ActivationFunctionType
A = mybir.AluOpType

N_WARM_MM = int(os.environ.get("K_WARM", "8"))  # PE HAM warm-up matmuls
SO_WAIT = os.environ.get("K_SOWAIT", "0") == "1"  # wait out-DMA completion
ZD_FP8 = os.environ.get("K_ZD8", "1") == "1"    # fp8 warm-up tile
HOIST_W1 = os.environ.get("K_HW1", "0") == "1"  # w1 wait inside warm-up
ZD_WAIT = os.environ.get("K_ZDWAIT", "0") == "1"  # PE waits for zd memset
SPLIT_STT = os.environ.get("K_SSTT", "1") == "1"  # wav in two k-halves
W1_ACT = os.environ.get("K_W1ACT", "1") == "1"  # issue w1 DMA from ACT queue
XC_SPLIT = os.environ.get("K_XCSPLIT", "0") == "1"  # xc as 2 partition-halves
# (measured slower: the extra ACT-queue issue delays w1 and the 64-row
# receipt latency does not halve — kept only as an A/B toggle)
DERF = os.environ.get("K_DERF", "1") == "1"  # e via Derivative_Erf from x
SPLIT_E = os.environ.get("K_SE", "1") == "1"  # e in two k-halves
# Derivative_Erf(x/sqrt2) = (2/sqrt(pi))*exp(-x^2/2): computes e directly
# from x on ACT, in parallel with DVE's u=x^2, removing the serial u->exp
# leg.  The 2/sqrt(pi) factor is folded into the weights on the host (and
# BatchNorm is invariant to it anyway).  Not implemented in CoreSim — use
# K_DERF=0 for simulator runs.

_programs: dict[str, bass.Bass] = {}


def _build_bshard(warm: bool = True, out16: bool = True) -> bass.Bass:
    """Batch-sharded fast path: per-core x'^T slice [128, NK*BS] fp16 and
    full fp16 weights [128, NK*O]; outputs the un-normalized y slice
    [BS, O] (BatchNorm runs on the host)."""
    ODT = FP16 if out16 else FP32
    nc = bass.Bass("TRN2", target_bir_lowering=False, debug=False,
                   num_devices=N_CORES)
    xc = nc.dram_tensor("xc", [KP, NK * BS], FP16, kind="ExternalInput").ap()
    wt = nc.dram_tensor("wt", [KP, NK * O], FP16, kind="ExternalInput").ap()
    yc = nc.dram_tensor("yc", [BS, O], ODT, kind="ExternalOutput").ap()

    xs = nc.alloc_sbuf_tensor("xs", [KP, NK * BS], FP16).ap()
    u = nc.alloc_sbuf_tensor("u", [KP, NK * BS], FP16).ap()
    e = nc.alloc_sbuf_tensor("e", [KP, NK * BS], FP16).ap()
    wav = nc.alloc_sbuf_tensor("wav", [KP, NK * BS], FP16).ap()
    ws = nc.alloc_sbuf_tensor("ws", [KP, NK * O], FP16).ap()
    out_sb = nc.alloc_sbuf_tensor("out_sb", [BS, O], ODT).ap()
    # fp8 keeps the warm-up tile's memset short (~270ns) so the PE can
    # start its HAM warm-up matmuls as early as possible
    zdt = mybir.dt.float8e4 if ZD_FP8 else FP16
    zd = nc.alloc_sbuf_tensor("zd", [KP, O], zdt).ap()
    scr = nc.alloc_sbuf_tensor("scr", [1, 3], FP32).ap()
    psum = nc.alloc_psum_tensor("psum", [BS, O], FP32).ap()
    pz = nc.alloc_psum_tensor("pz", [BS, O], FP32).ap()
    const0 = nc.const_aps.aps[(FP32, 0.0)]
    HW = NK * O // 2        # 1024: weight DMA split point (chunks 0-1 / 2-3)
    HO = O // 2             # 256: PSUM->SBUF copy split (ACT left, DVE right)

    with nc.Block(no_gpsimd_drain=True) as block, \
         nc.semaphore("sxc") as sxc, \
         nc.semaphore("sw1") as sw1, \
         nc.semaphore("sw2") as sw2, \
         nc.semaphore("sz") as sz, \
         nc.semaphore("su") as su, \
         nc.semaphore("se") as se, \
         nc.semaphore("sv") as sv, \
         nc.semaphore("spe") as spe, \
         nc.semaphore("sco") as sco, \
         nc.semaphore("so") as so:

        HP = KP // 2  # 64: xc partition-split point

        @block.sync
        def _(sp):
            if XC_SPLIT:
                # lower partition half; upper half rides the ACT queue so
                # the two 64-row descriptor gens and receipts run in
                # parallel (~0.35us earlier xc availability)
                sp.dma_start(out=xs[0:HP, :],
                             in_=xc[0:HP, :]).then_inc(sxc, 16)
            else:
                sp.dma_start(out=xs[:], in_=xc[:]).then_inc(sxc, 16)
            if not W1_ACT:
                sp.dma_start(out=ws[:, 0:HW],
                             in_=wt[:, 0:HW]).then_inc(sw1, 16)
            sp.dma_start(out=ws[:, HW:], in_=wt[:, HW:]).then_inc(sw2, 16)
            sp.wait_ge(sco, 1)
            sp.dma_start(out=yc[:], in_=out_sb[:]).then_inc(so, 16)
            if SO_WAIT:
                sp.wait_ge(so, 16)

        @block.gpsimd
        def _(gp):
            if warm:
                # zeros for the PE warm-up matmuls
                gp.memset(zd[:], 0.0).then_inc(sz)

        @block.scalar
        def _(act):
            if XC_SPLIT:
                act.dma_start(out=xs[HP:KP, :],
                              in_=xc[HP:KP, :]).then_inc(sxc, 16)
            if W1_ACT:
                # first weights half rides the ACT HWDGE queue so its
                # transfer overlaps xc/w2 on the SP queue
                act.dma_start(out=ws[:, 0:HW],
                              in_=wt[:, 0:HW]).then_inc(sw1, 16)
            # warmup activation triggers the one ACT table load at t~0
            # (must use the same table set as the real activation)
            EFUNC = F.Derivative_Erf if DERF else F.Exp
            act.activation(scr[0:1, 2:3], const0[0:1, :], EFUNC,
                           bias=0.0, scale=1.0)
            if DERF:
                # e ~ exp(-x^2/2) straight from x, parallel with DVE's u
                act.wait_ge(sxc, 32 if XC_SPLIT else 16)
                sc2 = float(1.0 / math.sqrt(2.0))
                if SPLIT_E:
                    HE = 2 * BS
                    act.activation(e[:, 0:HE], xs[:, 0:HE],
                                   F.Derivative_Erf, bias=0.0,
                                   scale=sc2).then_inc(se)
                    act.activation(e[:, HE:], xs[:, HE:],
                                   F.Derivative_Erf, bias=0.0,
                                   scale=sc2).then_inc(se)
                else:
                    act.activation(e[:], xs[:], F.Derivative_Erf, bias=0.0,
                                   scale=sc2).then_inc(se)
            else:
                act.wait_ge(su, 1)
                act.activation(e[:], u[:], F.Exp, bias=0.0,
                               scale=-0.5).then_inc(se)

        @block.vector
        def _(dve):
            HB = 2 * BS  # 128: wav split point (k-chunks 0-1 / 2-3)
            dve.wait_ge(sxc, 32 if XC_SPLIT else 16)
            dve.tensor_mul(u[:], xs[:], xs[:]).then_inc(su)
            dve.wait_ge(se, 1)
            if SPLIT_STT:
                # two halves so the k0/k1 matmuls can start ~0.3us earlier
                dve.scalar_tensor_tensor(out=wav[:, 0:HB], in0=u[:, 0:HB],
                                         scalar=1.0, in1=e[:, 0:HB],
                                         op0=A.subtract,
                                         op1=A.mult).then_inc(sv)
                if SPLIT_E:
                    dve.wait_ge(se, 2)
                dve.scalar_tensor_tensor(out=wav[:, HB:], in0=u[:, HB:],
                                         scalar=1.0, in1=e[:, HB:],
                                         op0=A.subtract,
                                         op1=A.mult).then_inc(sv)
            else:
                dve.scalar_tensor_tensor(out=wav[:], in0=u[:], scalar=1.0,
                                         in1=e[:], op0=A.subtract,
                                         op1=A.mult).then_inc(sv)
            # PSUM->SBUF(fp16) eviction: DVE only — ACT and DVE reading
            # disjoint halves of one PSUM bank concurrently wedges the HW
            dve.wait_ge(spe, 1)
            dve.tensor_copy(out_sb[:], psum[:]).then_inc(sco)

        @block.tensor
        def _(pe):
            if warm:
                # ~3.4us of dummy matmuls during the DMA wait flips the PE
                # HAM clock gate to full rate before the real matmuls issue.
                # No wait on the memset: reading zd before it lands only
                # feeds garbage into discarded dummy results, and skipping
                # the wait starts the HAM busy-window ~0.35us earlier.
                if ZD_WAIT:
                    pe.wait_ge(sz, 1)
                for _i in range(N_WARM_MM):
                    pe.matmul(pz[:], lhsT=zd[:, 0:BS], rhs=zd[:],
                              start=True, stop=True)
                    if HOIST_W1 and _i == min(3, N_WARM_MM - 1):
                        # w1 lands mid-warm-up; waiting here keeps the
                        # post-warm-up path down to the sv wait alone
                        pe.wait_ge(sw1, 16)
            pe.wait_ge(sv, 1)
            for k in range(NK):
                if k == 0 and not (warm and HOIST_W1):
                    pe.wait_ge(sw1, 16)
                elif k == 2:
                    if SPLIT_STT:
                        pe.wait_ge(sv, 2)
                    pe.wait_ge(sw2, 16)
                mm = pe.matmul(psum[:], lhsT=wav[:, k * BS:(k + 1) * BS],
                               rhs=ws[:, k * O:(k + 1) * O],
                               start=(k == 0), stop=(k == NK - 1))
                if k == NK - 1:
                    mm.then_inc(spe)
    return nc


BSHARD_WARM = True
BSHARD_OUT16 = True


def _get_program(name: str) -> bass.Bass:
    if name not in _programs:
        assert name == "bshard"
        _programs[name] = _build_bshard(warm=BSHARD_WARM,
                                        out16=BSHARD_OUT16)
    return _programs[name]


def _pack_k(v2d: np.ndarray) -> np.ndarray:
    """(I, C) -> (KP, NK*C): out[p, k*C:(k+1)*C] = v2d[k*KP+p, :]."""
    c = v2d.shape[1]
    return np.ascontiguousarray(
        v2d.reshape(NK, KP, c).transpose(1, 0, 2).reshape(KP, NK * c))


_last_results = None  # BassKernelResults of the most recent run (for test.py)
TRACE = False
TRACE_KW: dict = {}


def kernel(x, scale, bias, weight, gamma, beta):
    x = np.asarray(x, dtype=np.float32)
    scale = np.asarray(scale, dtype=np.float32)
    bias = np.asarray(bias, dtype=np.float32)
    # MEXHAT_NORM folded into the weights (device computes (t^2-1)e^{-t^2/2})
    weight = np.asarray(weight, dtype=np.float32) * np.float32(MEXHAT_NORM)
    gamma = np.asarray(gamma, dtype=np.float32)
    beta = np.asarray(beta, dtype=np.float32)
    assert x.shape == (B, I) and weight.shape == (I, O)

    global _last_results
    fast = bool(np.all(scale == scale[:, :1]) and np.all(bias == bias[:, :1]))
    if fast:
        # fold the (constant-along-O) affine into x on the host
        with np.errstate(divide="ignore", invalid="ignore"):
            xp = (x - bias[:, 0][None, :]) / scale[:, 0][None, :]
        fast = bool(np.all(np.isfinite(xp)) and np.abs(xp).max() < 6.0e4)

    if fast:
        # x'^T k-chunk packed: [128, NK*BS] per core; fp16 datapath
        xpT16 = np.ascontiguousarray(xp.T).astype(np.float16)  # (I, B)
        wdev = weight
        if DERF:
            # device computes e = (2/sqrt(pi))*exp(-u/2); fold the
            # sqrt(pi)/2 back into the weights
            wdev = weight * np.float32(math.sqrt(math.pi) / 2.0)
        wt16 = _pack_k(wdev).astype(np.float16)                # (KP, NK*O)
        in_maps = []
        for c in range(N_CORES):
            bsl = slice(c * BS, (c + 1) * BS)
            xc = np.ascontiguousarray(
                xpT16[:, bsl].reshape(NK, KP, BS)
                .transpose(1, 0, 2).reshape(KP, NK * BS))
            in_maps.append({"xc": xc, "wt": wt16})
        nc = _get_program("bshard")
        res = run_bass_kernel_spmd(nc, in_maps, list(range(N_CORES)),
                                   trace=TRACE, **TRACE_KW)
        _last_results = res
        y = np.empty((B, O), dtype=np.float64)
        for c in range(N_CORES):
            y[c * BS:(c + 1) * BS, :] = res.results[c]["yc"]
        # BatchNorm (training stats) epilogue on the host
        mean = y.mean(axis=0)
        var = ((y - mean) ** 2).mean(axis=0)
        out = (y - mean) / np.sqrt(var + BN_EPS) * gamma + beta
        return out.astype(np.float32)

    # general fallback (scale/bias vary along O): exact numpy evaluation.
    # The canonical inputs never hit this; it exists for correctness only.
    xd = x.astype(np.float64)
    sd = scale.astype(np.float64)
    bd = bias.astype(np.float64)
    wd = weight.astype(np.float64)  # MEXHAT_NORM already folded in
    y = np.zeros((B, O), dtype=np.float64)
    for i0 in range(0, I, 32):
        i1 = i0 + 32
        t = (xd[:, i0:i1, None] - bd[None, i0:i1, :]) / sd[None, i0:i1, :]
        t2 = t * t
        wav = (t2 - 1.0) * np.exp(-0.5 * t2)
        y += np.einsum("bio,io->bo", wav, wd[i0:i1, :])
    mean = y.mean(axis=0)
    var = ((y - mean) ** 2).mean(axis=0)
    out = (y - mean) / np.sqrt(var + BN_EPS) * gamma + beta
    return out.astype(np.float32)
